# revision 22
# baseline (speedup 1.0000x reference)
"""LoRA-MoE fused kernel for 8x Trainium2 NeuronCores (Bass/Tile).

Math (per batch sample b, data-parallel across 8 cores):
    g_b    = gate_w @ mean_s(x_b) + gate_b                      # [E]
    out_b  = x_b @ W^T + ((x_b @ A^T) * g_rep) @ Bt + bias      # [S, D_OUT]
where A = lora_A reshaped [E*R, D_IN], Bt[(e,r), o] = lora_B[e, o, r],
g_rep[(e,r)] = g_b[e].  Merged per-sample weights are never materialized.

Precision strategy (gate: rel_err < 2e-2):
  - K-chunks 0..KB-1 of the base GEMM run in bf16 (measured matmul pitch
    217 ns vs 228 ns f32r, and half the HBM traffic of f32).
  - The last KQ=2 K-chunks run as ONE fp8-e4m3 DoubleRow matmul
    (K=256 in 221 ns -> 2x per-MAC vs bf16).  e4m3 on 2/8 of K costs
    rel_err 0.036*sqrt(2/8) ~= 1.8e-2; everything is deterministic
    (host-side rounding, f32 PSUM accumulation), so the margin holds.
  - LoRA path (u = A@x, then bts^T @ u) in bf16; its output contribution
    is ~1.6e-3 of the total, so its quantization error is negligible.
  - Output stored fp16 (halves the store traffic; ~1e-4 error).

Schedule: out^T tiles [o_tile=128 part, s_chunk=512 free]; contraction
on partitions.  ot0 runs base-only first (its PSUM groups stay open),
then gate + u while x finishes landing, then ot0's lora is appended and
ot1..31 run fused base+lora, alternating 4-bank PSUM groups.  PSUM->SBUF
copies add the bias and alternate Vector/Scalar engines.  DMA is spread
over the sync/scalar/gpsimd rings with fine-grained early x pieces.
"""

import sys

import numpy as np
import ml_dtypes

try:
    import concourse.bass  # noqa: F401
except ImportError:  # pragma: no cover - fallback for bare environments
    for _p in (
        "/root/.axon_site",
        "/root/.axon_site/_ro/trn_rl_repo",
        "/root/.axon_site/_ro/pypackages",
        "/opt/trn_rl_repo",
    ):
        if _p not in sys.path:
            sys.path.append(_p)

import concourse.bass as bass  # noqa: F401
import concourse.mybir as mybir
import concourse.tile as tile
from concourse import bacc, bass_utils

S, B, D_IN, D_OUT, E, R = 2048, 8, 1024, 4096, 8, 16
NCORES = 8
ER = E * R            # 128 (one partition dim worth of lora rows)
KC = D_IN // 128      # 8 contraction chunks
NOT = D_OUT // 128    # 32 output tiles
SC = 512              # s-chunk (one PSUM bank of f32)
NSC = S // SC         # 4

KQ = 2                # trailing K-chunks in fp8-e4m3 via DoubleRow
KB = KC - KQ          # leading K-chunks in bf16
NDEF = 2              # deferred o_tiles (base before the gate is ready)

F32 = mybir.dt.float32
BF16 = mybir.dt.bfloat16
FP16 = mybir.dt.float16
FP8 = mybir.dt.float8e4
DRMODE = mybir.MatmulPerfMode.DoubleRow

NP_BF16 = ml_dtypes.bfloat16
NP_E4M3 = ml_dtypes.float8_e4m3

Ident = mybir.ActivationFunctionType.Identity


def _build_nc(n_cores: int = NCORES):
    nc = bacc.Bacc(
        "TRN2", target_bir_lowering=False, debug=False, num_devices=n_cores
    )

    xb_d = nc.dram_tensor("xb", [128, KB, S], BF16, kind="ExternalInput").ap()
    wte_d = nc.dram_tensor(
        "wte", [128, KB, NDEF, 128], BF16, kind="ExternalInput"
    ).ap()
    wp_b = nc.dram_tensor(
        "wp_b", [128, KB, NOT - NDEF, 128], BF16, kind="ExternalInput"
    ).ap()
    at_b_d = nc.dram_tensor("at_b", [128, KB, ER], BF16, kind="ExternalInput").ap()
    bt_d = nc.dram_tensor("bt", [ER, D_OUT], BF16, kind="ExternalInput").ap()
    gw_d = nc.dram_tensor("gw", [128, KC, ER], BF16, kind="ExternalInput").ap()
    gb_d = nc.dram_tensor("gb", [ER, 1], F32, kind="ExternalInput").ap()
    bias_d = nc.dram_tensor("bias_t", [128, NOT], F32, kind="ExternalInput").ap()
    if KQ:
        xq_d = nc.dram_tensor("xq", [128, KQ, S], FP8, kind="ExternalInput").ap()
        wq_d = nc.dram_tensor(
            "wq_all", [128, KQ, NOT, 128], FP8, kind="ExternalInput"
        ).ap()
        at_q_d = nc.dram_tensor("at_q", [128, KQ, ER], FP8, kind="ExternalInput").ap()
    outT = nc.dram_tensor("outT", [D_OUT, S], FP16, kind="ExternalOutput").ap()

    with (
        tile.TileContext(nc) as tc,
        tc.tile_pool(name="singles", bufs=1) as singles,
        tc.tile_pool(name="opool", bufs=5) as opool,
        tc.tile_pool(name="ps_a", bufs=4, space="PSUM") as ps_a,
        tc.tile_pool(name="ps_b", bufs=4, space="PSUM") as ps_b,
    ):
        # ---- resident SBUF tensors
        x_sb = singles.tile([128, KB, S], BF16)
        wte_sb = singles.tile([128, KB, NDEF, 128], BF16)
        w_sb = singles.tile([128, KB, NOT - NDEF, 128], BF16)
        at_sb = singles.tile([128, KB, ER], BF16)
        bt_sb = singles.tile([128, D_OUT], BF16)
        gw_sb = singles.tile([128, KC, ER], BF16)
        gb_sb = singles.tile([128, 1], F32)
        bias_sb = singles.tile([128, NOT], F32)
        if KQ:
            xq_sb = singles.tile([128, KQ, S], FP8)
            wq_sb = singles.tile([128, KQ, NOT, 128], FP8)
            atq_sb = singles.tile([128, KQ, ER], FP8)
        xsum = singles.tile([128, KC], BF16)
        u_sb = singles.tile([128, S], BF16)
        g_sb = singles.tile([128, 1], F32)
        bts_sb = singles.tile([128, D_OUT], BF16)

        # ---- DMA kickoff.  Rings issue packets serially per queue and the
        # aggregate HBM read rate is the startup constraint, so the first
        # ~20us carries ONLY the working set of the deferred phase (wte + x
        # chunks + small gate tensors); all bulk weight loads queue strictly
        # behind them on their rings.  x chunk 0 is partition-split across
        # two rings for first-matmul latency.
        # Ring rates are uneven under contention and the gpsimd queue starts
        # ~4us late, so every x chunk is partition-split across two rings
        # round-robin: stream completion then tracks the aggregate HBM rate
        # instead of the slowest ring.  gpsimd gets only later-needed halves.
        # Whole-chunk x DMAs (halving every chunk across rings measured ~4.5us
        # slower).  Only x chunk 0 and wte are partition-split for first-
        # matmul latency.  xq leads the gpsimd queue (which starts ~4us late
        # but is otherwise idle): late xq cascades through xsum -> gate ->
        # bts into multi-us PE gaps.  wq/bt are split so they land by their
        # deadlines (~20us / ~26us) without racing the x stream.
        HP = 64
        nc.sync.dma_start(out=wte_sb[:HP], in_=wte_d[:HP])
        nc.sync.dma_start(out=x_sb[:HP, 0, :], in_=xb_d[:HP, 0, :])
        nc.sync.dma_start(out=x_sb[:, 1, :], in_=xb_d[:, 1, :])
        nc.sync.dma_start(out=x_sb[:, 4, :], in_=xb_d[:, 4, :])
        nc.sync.dma_start(out=gb_sb[:], in_=gb_d)
        nc.sync.dma_start(out=bias_sb[:], in_=bias_d)
        nc.sync.dma_start(out=bt_sb[HP:], in_=bt_d[HP:])
        nc.scalar.dma_start(out=wte_sb[HP:], in_=wte_d[HP:])
        nc.scalar.dma_start(out=x_sb[HP:, 0, :], in_=xb_d[HP:, 0, :])
        nc.scalar.dma_start(out=x_sb[:, 2, :], in_=xb_d[:, 2, :])
        nc.scalar.dma_start(out=x_sb[:, 5, :], in_=xb_d[:, 5, :])
        if KQ:
            nc.scalar.dma_start(out=wq_sb[:HP], in_=wq_d[:HP])
        nc.scalar.dma_start(out=bt_sb[:HP], in_=bt_d[:HP])
        if KQ:
            nc.gpsimd.dma_start(out=xq_sb[:], in_=xq_d[:])
        nc.gpsimd.dma_start(out=x_sb[:, 3, :], in_=xb_d[:, 3, :])
        if KQ:
            nc.gpsimd.dma_start(out=wq_sb[HP:], in_=wq_d[HP:])
        nc.gpsimd.dma_start(out=at_sb[:], in_=at_b_d)
        if KQ:
            nc.gpsimd.dma_start(out=atq_sb[:], in_=at_q_d)
        nc.gpsimd.dma_start(out=gw_sb[:], in_=gw_d)
        for c in range(3):
            nc.sync.dma_start(out=w_sb[:, c], in_=wp_b[:, c])
        nc.gpsimd.dma_start(out=w_sb[:, 3], in_=wp_b[:, 3])
        nc.scalar.dma_start(out=w_sb[:, 4], in_=wp_b[:, 4])
        nc.scalar.dma_start(out=w_sb[:, 5], in_=wp_b[:, 5])

        # ---- column sums for the gate, as x chunks land (all on Vector).
        # bf16 output is fine: DVE accumulates in f32 internally and the
        # gate path tolerates ~1e-2 relative error (its output share is
        # ~1.6e-3 of the total).
        with nc.allow_low_precision(reason="gate xsum, f32 internal accum"):
            for c in range(KQ):  # slow fp8 reduces first (xq lands early)
                nc.vector.reduce_sum(
                    out=xsum[:, KB + c : KB + c + 1],
                    in_=xq_sb[:, c, :],
                    axis=mybir.AxisListType.X,
                )
            for c in range(KB):
                nc.vector.reduce_sum(
                    out=xsum[:, c : c + 1],
                    in_=x_sb[:, c, :],
                    axis=mybir.AxisListType.X,
                )

        def wsl(ot, c):
            if ot < NDEF:
                return wte_sb[:, c, ot, :]
            return w_sb[:, c, ot - NDEF, :]

        def wqsl(ot):
            return wq_sb[:, :, ot, :]

        _ps_toggle = [0]

        def psum_group():
            pool = ps_a if _ps_toggle[0] % 2 == 0 else ps_b
            _ps_toggle[0] += 1
            accs = []
            for _sc in range(NSC):
                acc = pool.tile([128, SC], F32, tag="acc")
                accs.append(acc)
            return accs

        def base_mms(accs, ot, close):
            for c in range(KB):
                for sc in range(NSC):
                    nc.tensor.matmul(
                        accs[sc][:],
                        wsl(ot, c),
                        x_sb[:, c, sc * SC : (sc + 1) * SC],
                        start=(c == 0),
                        stop=False,
                    )
            if KQ:
                for sc in range(NSC):
                    nc.tensor.matmul(
                        accs[sc][:],
                        wqsl(ot),
                        xq_sb[:, :, sc * SC : (sc + 1) * SC],
                        start=False,
                        stop=False,
                        perf_mode=DRMODE,
                    )
            if close:
                lora_mms(accs, ot)

        def lora_mms(accs, ot):
            osl = slice(ot * 128, (ot + 1) * 128)
            for sc in range(NSC):
                nc.tensor.matmul(
                    accs[sc][:],
                    bts_sb[:, osl],
                    u_sb[:, sc * SC : (sc + 1) * SC],
                    start=False,
                    stop=True,
                )

        def bias_copy(o_sb, accs, ot):
            for sc in range(NSC):
                sl = slice(sc * SC, (sc + 1) * SC)
                if (ot + sc) % 2 == 0:
                    nc.vector.tensor_scalar_add(
                        o_sb[:, sl], accs[sc][:], bias_sb[:, ot : ot + 1]
                    )
                else:
                    nc.scalar.activation(
                        out=o_sb[:, sl],
                        in_=accs[sc][:],
                        func=Ident,
                        bias=bias_sb[:, ot : ot + 1],
                        scale=1.0,
                    )

        # ---- ot0 + ot1: base-only, interleaved chunk-major so the PE's
        # consumption rate (~1.74us/chunk for 8 MMs) matches the x DMA
        # arrival rate.  Groups close after the DR chunk; the lora part is
        # patched in later via separate PSUM groups + DVE adds.
        accs0 = psum_group()          # ps_a
        accs1 = psum_group()          # ps_b
        both = (accs0, accs1)
        for c in range(KB):
            for oti in range(2):
                for sc in range(NSC):
                    nc.tensor.matmul(
                        both[oti][sc][:],
                        wsl(oti, c),
                        x_sb[:, c, sc * SC : (sc + 1) * SC],
                        start=(c == 0),
                        stop=(KQ == 0 and c == KB - 1),
                    )
        if KQ:
            for oti in range(2):
                for sc in range(NSC):
                    nc.tensor.matmul(
                        both[oti][sc][:],
                        wqsl(oti),
                        xq_sb[:, :, sc * SC : (sc + 1) * SC],
                        start=False,
                        stop=True,
                        perf_mode=DRMODE,
                    )
        o_defer = []
        for oti in range(2):
            o_sb = opool.tile([128, S], FP16, tag="od")
            bias_copy(o_sb, both[oti], oti)
            o_defer.append(o_sb)

        # ---- u^T[er, s] = A @ x_b^T  (bf16 + fp8-DR chunks, 2 banks
        # ping-pong from the freed ps_a slots)
        ub0 = ps_a.tile([128, SC], F32, tag="acc")
        ub1 = ps_a.tile([128, SC], F32, tag="acc")
        for sc in range(NSC):
            up = ub0 if sc % 2 == 0 else ub1
            sl = slice(sc * SC, (sc + 1) * SC)
            for c in range(KB):
                nc.tensor.matmul(
                    up[:],
                    at_sb[:, c, :],
                    x_sb[:, c, sl],
                    start=(c == 0),
                    stop=(KQ == 0 and c == KB - 1),
                )
            if KQ:
                nc.tensor.matmul(
                    up[:],
                    atq_sb[:],
                    xq_sb[:, :, sl],
                    start=False,
                    stop=True,
                    perf_mode=DRMODE,
                )
            nc.vector.tensor_copy(u_sb[:, sl], up[:])

        # ---- gate (after u: xsum reduces are off the critical path):
        # g[er] = sum_c gw[:,c,:]^T @ xsum[:,c] + gb  (gw pre-scaled 1/S)
        g_ps = ps_b.tile([128, 1], F32, tag="acc")
        for c in range(KC):
            nc.tensor.matmul(
                g_ps[:],
                gw_sb[:, c, :],
                xsum[:, c : c + 1],
                start=(c == 0),
                stop=(c == KC - 1),
            )
        nc.vector.tensor_add(g_sb[:], g_ps[:], gb_sb[:])

        # fold the gate into Bt: bts[er, o] = g[er] * Bt[er, o],
        # split across Vector and Scalar engines
        half = D_OUT // 2
        nc.scalar.activation(
            out=bts_sb[:, :half], in_=bt_sb[:, :half], func=Ident, scale=g_sb[:]
        )
        nc.vector.tensor_scalar_mul(bts_sb[:, half:], bt_sb[:, half:], g_sb[:])

        def lora_patch(oti, pool):
            laccs = []
            for _sc in range(NSC):
                lacc = pool.tile([128, SC], F32, tag="acc")
                laccs.append(lacc)
            osl = slice(oti * 128, (oti + 1) * 128)
            for sc in range(NSC):
                nc.tensor.matmul(
                    laccs[sc][:],
                    bts_sb[:, osl],
                    u_sb[:, sc * SC : (sc + 1) * SC],
                    start=True,
                    stop=True,
                )
            with nc.allow_low_precision(reason="lora add into fp16 out tile"):
                for sc in range(NSC):
                    sl = slice(sc * SC, (sc + 1) * SC)
                    nc.vector.tensor_add(
                        o_defer[oti][:, sl], o_defer[oti][:, sl], laccs[sc][:]
                    )
            ring = nc.sync if oti % 2 == 0 else nc.gpsimd
            ring.dma_start(out=outT[osl, :], in_=o_defer[oti][:])

        # ---- steady-state fused loop, lora patches for ot0/ot1 slotted
        # between ot2 and ot3 (bts/u are ready well before then)
        for ot in range(2, NOT):
            o_sb = opool.tile([128, S], FP16, tag="o")
            osl = slice(ot * 128, (ot + 1) * 128)
            accs = psum_group()
            if ot == NOT - 1:
                # drain the tail per s-chunk: lora -> copy-halves on both
                # engines -> store piece, so the last copies don't stack up
                # after the last matmul
                base_mms(accs, ot, close=False)
                for sc in range(NSC):
                    sl0 = slice(sc * SC, sc * SC + SC // 2)
                    sl1 = slice(sc * SC + SC // 2, (sc + 1) * SC)
                    nc.tensor.matmul(
                        accs[sc][:],
                        bts_sb[:, osl],
                        u_sb[:, sc * SC : (sc + 1) * SC],
                        start=False,
                        stop=True,
                    )
                    nc.vector.tensor_scalar_add(
                        o_sb[:, sl0], accs[sc][:, : SC // 2], bias_sb[:, ot : ot + 1]
                    )
                    nc.scalar.activation(
                        out=o_sb[:, sl1],
                        in_=accs[sc][:, SC // 2 :],
                        func=Ident,
                        bias=bias_sb[:, ot : ot + 1],
                        scale=1.0,
                    )
                    ring = nc.sync if sc % 2 == 0 else nc.gpsimd
                    ring.dma_start(
                        out=outT[osl, sc * SC : (sc + 1) * SC],
                        in_=o_sb[:, sc * SC : (sc + 1) * SC],
                    )
                continue
            base_mms(accs, ot, close=True)
            bias_copy(o_sb, accs, ot)
            if ot == 2:
                lora_patch(0, ps_b)
            elif ot == 3:
                lora_patch(1, ps_a)
            if ot == NOT - 2:
                for sc in range(NSC):
                    sl = slice(sc * SC, (sc + 1) * SC)
                    ring = nc.sync if (ot + sc) % 2 == 0 else nc.gpsimd
                    ring.dma_start(out=outT[osl, sl], in_=o_sb[:, sl])
            else:
                ring = nc.sync if ot % 2 == 0 else nc.gpsimd
                ring.dma_start(out=outT[osl, :], in_=o_sb[:])

    nc.compile()
    return nc


def _prep_in_maps(x, gate_w, gate_b, W, bias, lora_A, lora_B):
    f32 = np.float32
    x = np.asarray(x, f32)
    gate_w = np.asarray(gate_w, f32)
    gate_b = np.asarray(gate_b, f32)
    W = np.asarray(W, f32)
    bias = np.asarray(bias, f32)
    lora_A = np.asarray(lora_A, f32)
    lora_B = np.asarray(lora_B, f32)

    # WTb[ot, p, c, m] = W[ot*128 + m, c*128 + p]
    WTb = W.reshape(NOT, 128, KC, 128).transpose(0, 3, 2, 1)
    wte = np.ascontiguousarray(WTb[:NDEF, :, :KB, :].transpose(1, 2, 0, 3)).astype(
        NP_BF16
    )
    wp_b = np.ascontiguousarray(WTb[NDEF:, :, :KB, :].transpose(1, 2, 0, 3)).astype(
        NP_BF16
    )
    # AT[p, c, er] = A[er, c*128 + p]
    AT = lora_A.reshape(ER, D_IN).T.reshape(KC, 128, ER).transpose(1, 0, 2)
    at_b = np.ascontiguousarray(AT[:, :KB, :]).astype(NP_BF16)
    Bt = np.ascontiguousarray(lora_B.transpose(0, 2, 1).reshape(ER, D_OUT)).astype(
        NP_BF16
    )
    gwT = (
        (np.repeat(gate_w, R, axis=0).T / np.float32(S))
        .reshape(KC, 128, ER)
        .transpose(1, 0, 2)
    )
    gw = np.ascontiguousarray(gwT).astype(NP_BF16)
    gbr = np.ascontiguousarray(np.repeat(gate_b, R).reshape(ER, 1))
    bias_t = np.ascontiguousarray(bias.reshape(NOT, 128).T)

    shared = {
        "wte": wte,
        "wp_b": wp_b,
        "at_b": at_b,
        "bt": Bt,
        "gw": gw,
        "gb": gbr,
        "bias_t": bias_t,
    }
    if KQ:
        shared["wq_all"] = np.ascontiguousarray(
            WTb[:, :, KB:, :].transpose(1, 2, 0, 3)
        ).astype(NP_E4M3)
        shared["at_q"] = np.ascontiguousarray(AT[:, KB:, :]).astype(NP_E4M3)

    in_maps = []
    for b in range(NCORES):
        m = dict(shared)
        # p-major: xb[p, c, s] = x[s, b, c*128 + p], so each DMA line is a
        # full 4 KB row per partition
        xT = x[:, b, :].T.reshape(KC, 128, S)
        m["xb"] = np.ascontiguousarray(xT[:KB].transpose(1, 0, 2)).astype(NP_BF16)
        if KQ:
            m["xq"] = np.ascontiguousarray(xT[KB:].transpose(1, 0, 2)).astype(
                NP_E4M3
            )
        in_maps.append(m)
    return in_maps


def run(inputs, trace=False, trace_cores=None):
    """Build + run on 8 cores. Returns (out [S,B,D_OUT], BassKernelResults)."""
    in_maps = _prep_in_maps(**inputs)
    nc = _build_nc()
    kwargs = {}
    if trace:
        _register_axon_ntff_hook()
        kwargs = dict(trace=True, trace_cores=trace_cores or [0])
    res = bass_utils.run_bass_kernel_spmd(
        nc, in_maps, core_ids=list(range(NCORES)), **kwargs
    )
    out = np.empty((S, B, D_OUT), np.float32)
    for b in range(NCORES):
        out[:, b, :] = res.results[b]["outT"].T.astype(np.float32)
    return out, res


def _register_axon_ntff_hook():
    """antenv.axon_hooks is missing on this image; synthesize it so
    run_bass_kernel_spmd(trace=True) can reach the axon NTFF profiler."""
    import types

    try:
        from antenv.axon_hooks import get_axon_ntff_profile_hook  # noqa: F401

        return  # real module present
    except ImportError:
        pass
    try:
        from trn_agent_boot.trn_boot import _ntff_profile_via_ctypes
    except ImportError:
        return
    import antenv

    mod = types.ModuleType("antenv.axon_hooks")
    _state = {"hook": None}
    mod.set_axon_ntff_profile_hook = lambda h: _state.__setitem__("hook", h)
    mod.get_axon_ntff_profile_hook = lambda: _state["hook"]
    sys.modules["antenv.axon_hooks"] = mod
    antenv.axon_hooks = mod
    hook = _ntff_profile_via_ctypes("/opt/axon/libaxon_pjrt.so")
    if hook is not None:
        mod.set_axon_ntff_profile_hook(hook)


def kernel(**inputs) -> np.ndarray:
    out, _ = run(inputs, trace=False)
    return out


# revision 23
# speedup vs baseline: 1.0256x; 1.0256x over previous
"""LoRA-MoE fused kernel for 8x Trainium2 NeuronCores (Bass/Tile).

Math (per batch sample b, data-parallel across 8 cores):
    g_b    = gate_w @ mean_s(x_b) + gate_b                      # [E]
    out_b  = x_b @ W^T + ((x_b @ A^T) * g_rep) @ Bt + bias      # [S, D_OUT]
where A = lora_A reshaped [E*R, D_IN], Bt[(e,r), o] = lora_B[e, o, r],
g_rep[(e,r)] = g_b[e].  Merged per-sample weights are never materialized.

Precision strategy (gate: rel_err < 2e-2):
  - K-chunks 0..KB-1 of the base GEMM run in bf16 (measured matmul pitch
    217 ns vs 228 ns f32r, and half the HBM traffic of f32).
  - The last KQ=2 K-chunks run as ONE fp8-e4m3 DoubleRow matmul
    (K=256 in 221 ns -> 2x per-MAC vs bf16).  e4m3 on 2/8 of K costs
    rel_err 0.036*sqrt(2/8) ~= 1.8e-2; everything is deterministic
    (host-side rounding, f32 PSUM accumulation), so the margin holds.
  - LoRA path (u = A@x, then bts^T @ u) in bf16; its output contribution
    is ~1.6e-3 of the total, so its quantization error is negligible.
  - Output stored fp16 (halves the store traffic; ~1e-4 error).

Schedule: out^T tiles [o_tile=128 part, s_chunk=512 free]; contraction
on partitions.  ot0 runs base-only first (its PSUM groups stay open),
then gate + u while x finishes landing, then ot0's lora is appended and
ot1..31 run fused base+lora, alternating 4-bank PSUM groups.  PSUM->SBUF
copies add the bias and alternate Vector/Scalar engines.  DMA is spread
over the sync/scalar/gpsimd rings with fine-grained early x pieces.
"""

import sys

import numpy as np
import ml_dtypes

try:
    import concourse.bass  # noqa: F401
except ImportError:  # pragma: no cover - fallback for bare environments
    for _p in (
        "/root/.axon_site",
        "/root/.axon_site/_ro/trn_rl_repo",
        "/root/.axon_site/_ro/pypackages",
        "/opt/trn_rl_repo",
    ):
        if _p not in sys.path:
            sys.path.append(_p)

import concourse.bass as bass  # noqa: F401
import concourse.mybir as mybir
import concourse.tile as tile
from concourse import bacc, bass_utils

S, B, D_IN, D_OUT, E, R = 2048, 8, 1024, 4096, 8, 16
NCORES = 8
ER = E * R            # 128 (one partition dim worth of lora rows)
KC = D_IN // 128      # 8 contraction chunks
NOT = D_OUT // 128    # 32 output tiles
SC = 512              # s-chunk (one PSUM bank of f32)
NSC = S // SC         # 4

KQ = 2                # trailing K-chunks in fp8-e4m3 via DoubleRow
KB = KC - KQ          # leading K-chunks in bf16
NDEF = 2              # deferred o_tiles (base before the gate is ready)

F32 = mybir.dt.float32
BF16 = mybir.dt.bfloat16
FP16 = mybir.dt.float16
FP8 = mybir.dt.float8e4
DRMODE = mybir.MatmulPerfMode.DoubleRow

NP_BF16 = ml_dtypes.bfloat16
NP_E4M3 = ml_dtypes.float8_e4m3

Ident = mybir.ActivationFunctionType.Identity


def _build_nc(n_cores: int = NCORES):
    nc = bacc.Bacc(
        "TRN2", target_bir_lowering=False, debug=False, num_devices=n_cores
    )

    xb_d = nc.dram_tensor("xb", [128, KB, S], BF16, kind="ExternalInput").ap()
    wte_d = nc.dram_tensor(
        "wte", [128, KB, NDEF, 128], BF16, kind="ExternalInput"
    ).ap()
    wp_b = nc.dram_tensor(
        "wp_b", [128, KB, NOT - NDEF, 128], BF16, kind="ExternalInput"
    ).ap()
    at_b_d = nc.dram_tensor("at_b", [128, KB, ER], BF16, kind="ExternalInput").ap()
    bt_d = nc.dram_tensor("bt", [ER, D_OUT], BF16, kind="ExternalInput").ap()
    gw_d = nc.dram_tensor("gw", [128, KC, ER], BF16, kind="ExternalInput").ap()
    gb_d = nc.dram_tensor("gb", [ER, 1], F32, kind="ExternalInput").ap()
    bias_d = nc.dram_tensor("bias_t", [128, NOT], F32, kind="ExternalInput").ap()
    if KQ:
        xq_d = nc.dram_tensor("xq", [128, KQ, S], FP8, kind="ExternalInput").ap()
        wq_d = nc.dram_tensor(
            "wq_all", [128, KQ, NOT, 128], FP8, kind="ExternalInput"
        ).ap()
        at_q_d = nc.dram_tensor("at_q", [128, KQ, ER], FP8, kind="ExternalInput").ap()
    outT = nc.dram_tensor("outT", [D_OUT, S], FP16, kind="ExternalOutput").ap()

    with (
        tile.TileContext(nc) as tc,
        tc.tile_pool(name="singles", bufs=1) as singles,
        tc.tile_pool(name="opool", bufs=5) as opool,
        tc.tile_pool(name="ps_a", bufs=4, space="PSUM") as ps_a,
        tc.tile_pool(name="ps_b", bufs=4, space="PSUM") as ps_b,
    ):
        # ---- resident SBUF tensors
        x_sb = singles.tile([128, KB, S], BF16)
        wte_sb = singles.tile([128, KB, NDEF, 128], BF16)
        w_sb = singles.tile([128, KB, NOT - NDEF, 128], BF16)
        at_sb = singles.tile([128, KB, ER], BF16)
        bt_sb = singles.tile([128, D_OUT], BF16)
        gw_sb = singles.tile([128, KC, ER], BF16)
        gb_sb = singles.tile([128, 1], F32)
        bias_sb = singles.tile([128, NOT], F32)
        if KQ:
            xq_sb = singles.tile([128, KQ, S], FP8)
            wq_sb = singles.tile([128, KQ, NOT, 128], FP8)
            atq_sb = singles.tile([128, KQ, ER], FP8)
        xsum = singles.tile([128, KC], BF16)
        u_sb = singles.tile([128, S], BF16)
        g_sb = singles.tile([128, 1], F32)
        bts_sb = singles.tile([128, D_OUT], BF16)

        # ---- DMA kickoff.  Rings issue packets serially per queue and the
        # aggregate HBM read rate is the startup constraint, so the first
        # ~20us carries ONLY the working set of the deferred phase (wte + x
        # chunks + small gate tensors); all bulk weight loads queue strictly
        # behind them on their rings.  x chunk 0 is partition-split across
        # two rings for first-matmul latency.
        # Ring rates are uneven under contention and the gpsimd queue starts
        # ~4us late, so every x chunk is partition-split across two rings
        # round-robin: stream completion then tracks the aggregate HBM rate
        # instead of the slowest ring.  gpsimd gets only later-needed halves.
        # Whole-chunk x DMAs (halving every chunk across rings measured ~4.5us
        # slower).  Only x chunk 0 and wte are partition-split for first-
        # matmul latency.  xq leads the gpsimd queue (which starts ~4us late
        # but is otherwise idle): late xq cascades through xsum -> gate ->
        # bts into multi-us PE gaps.  wq/bt are split so they land by their
        # deadlines (~20us / ~26us) without racing the x stream.
        # CRITICAL: each engine recycles a small DMA-semaphore pool, so the
        # 5th+ dma_start on a queue BLOCKS that engine's instruction stream
        # until an earlier DMA completes.  The scalar queue therefore gets
        # only 5 early loads (its ACTIVATE copies must start by ~13us);
        # sync/gpsimd absorb all bulk, where trigger stalls are harmless.
        HP = 64
        nc.sync.dma_start(out=wte_sb[:HP], in_=wte_d[:HP])
        nc.sync.dma_start(out=x_sb[:HP, 0, :], in_=xb_d[:HP, 0, :])
        nc.sync.dma_start(out=x_sb[:, 1, :], in_=xb_d[:, 1, :])
        nc.sync.dma_start(out=x_sb[:, 4, :], in_=xb_d[:, 4, :])
        nc.sync.dma_start(out=gb_sb[:], in_=gb_d)
        nc.sync.dma_start(out=bias_sb[:], in_=bias_d)
        nc.scalar.dma_start(out=wte_sb[HP:], in_=wte_d[HP:])
        nc.scalar.dma_start(out=x_sb[HP:, 0, :], in_=xb_d[HP:, 0, :])
        nc.scalar.dma_start(out=x_sb[:, 2, :], in_=xb_d[:, 2, :])
        nc.scalar.dma_start(out=x_sb[:, 5, :], in_=xb_d[:, 5, :])
        nc.scalar.dma_start(out=w_sb[:, 5], in_=wp_b[:, 5])
        if KQ:
            nc.gpsimd.dma_start(out=xq_sb[:], in_=xq_d[:])
        nc.gpsimd.dma_start(out=x_sb[:, 3, :], in_=xb_d[:, 3, :])
        if KQ:
            nc.gpsimd.dma_start(out=wq_sb[:], in_=wq_d[:])
        nc.gpsimd.dma_start(out=at_sb[:], in_=at_b_d)
        if KQ:
            nc.gpsimd.dma_start(out=atq_sb[:], in_=at_q_d)
        nc.gpsimd.dma_start(out=gw_sb[:], in_=gw_d)
        nc.gpsimd.dma_start(out=bt_sb[:], in_=bt_d)
        nc.gpsimd.dma_start(out=w_sb[:, 3], in_=wp_b[:, 3])
        for c in range(3):
            nc.sync.dma_start(out=w_sb[:, c], in_=wp_b[:, c])
        nc.sync.dma_start(out=w_sb[:, 4], in_=wp_b[:, 4])

        # ---- column sums for the gate, as x chunks land (all on Vector).
        # bf16 output is fine: DVE accumulates in f32 internally and the
        # gate path tolerates ~1e-2 relative error (its output share is
        # ~1.6e-3 of the total).
        with nc.allow_low_precision(reason="gate xsum, f32 internal accum"):
            for c in range(KQ):  # slow fp8 reduces first (xq lands early)
                nc.vector.reduce_sum(
                    out=xsum[:, KB + c : KB + c + 1],
                    in_=xq_sb[:, c, :],
                    axis=mybir.AxisListType.X,
                )
            for c in range(KB):
                nc.vector.reduce_sum(
                    out=xsum[:, c : c + 1],
                    in_=x_sb[:, c, :],
                    axis=mybir.AxisListType.X,
                )

        def wsl(ot, c):
            if ot < NDEF:
                return wte_sb[:, c, ot, :]
            return w_sb[:, c, ot - NDEF, :]

        def wqsl(ot):
            return wq_sb[:, :, ot, :]

        _ps_toggle = [0]

        def psum_group():
            pool = ps_a if _ps_toggle[0] % 2 == 0 else ps_b
            _ps_toggle[0] += 1
            accs = []
            for _sc in range(NSC):
                acc = pool.tile([128, SC], F32, tag="acc")
                accs.append(acc)
            return accs

        def base_mms(accs, ot, close):
            for c in range(KB):
                for sc in range(NSC):
                    nc.tensor.matmul(
                        accs[sc][:],
                        wsl(ot, c),
                        x_sb[:, c, sc * SC : (sc + 1) * SC],
                        start=(c == 0),
                        stop=False,
                    )
            if KQ:
                for sc in range(NSC):
                    nc.tensor.matmul(
                        accs[sc][:],
                        wqsl(ot),
                        xq_sb[:, :, sc * SC : (sc + 1) * SC],
                        start=False,
                        stop=False,
                        perf_mode=DRMODE,
                    )
            if close:
                lora_mms(accs, ot)

        def lora_mms(accs, ot):
            osl = slice(ot * 128, (ot + 1) * 128)
            for sc in range(NSC):
                nc.tensor.matmul(
                    accs[sc][:],
                    bts_sb[:, osl],
                    u_sb[:, sc * SC : (sc + 1) * SC],
                    start=False,
                    stop=True,
                )

        def bias_copy(o_sb, accs, ot):
            for sc in range(NSC):
                sl = slice(sc * SC, (sc + 1) * SC)
                if (ot + sc) % 2 == 0:
                    nc.vector.tensor_scalar_add(
                        o_sb[:, sl], accs[sc][:], bias_sb[:, ot : ot + 1]
                    )
                else:
                    nc.scalar.activation(
                        out=o_sb[:, sl],
                        in_=accs[sc][:],
                        func=Ident,
                        bias=bias_sb[:, ot : ot + 1],
                        scale=1.0,
                    )

        # ---- ot0 + ot1: base-only, interleaved chunk-major so the PE's
        # consumption rate (~1.74us/chunk for 8 MMs) matches the x DMA
        # arrival rate.  Groups close after the DR chunk; the lora part is
        # patched in later via separate PSUM groups + DVE adds.
        accs0 = psum_group()          # ps_a
        accs1 = psum_group()          # ps_b
        both = (accs0, accs1)
        for c in range(KB):
            for oti in range(2):
                for sc in range(NSC):
                    nc.tensor.matmul(
                        both[oti][sc][:],
                        wsl(oti, c),
                        x_sb[:, c, sc * SC : (sc + 1) * SC],
                        start=(c == 0),
                        stop=(KQ == 0 and c == KB - 1),
                    )
        if KQ:
            for oti in range(2):
                for sc in range(NSC):
                    nc.tensor.matmul(
                        both[oti][sc][:],
                        wqsl(oti),
                        xq_sb[:, :, sc * SC : (sc + 1) * SC],
                        start=False,
                        stop=True,
                        perf_mode=DRMODE,
                    )
        o_defer = []
        for oti in range(2):
            o_sb = opool.tile([128, S], FP16, tag="od")
            bias_copy(o_sb, both[oti], oti)
            o_defer.append(o_sb)

        # ---- u^T[er, s] = A @ x_b^T  (bf16 + fp8-DR chunks, 2 banks
        # ping-pong from the freed ps_a slots)
        ub0 = ps_a.tile([128, SC], F32, tag="acc")
        ub1 = ps_a.tile([128, SC], F32, tag="acc")
        for sc in range(NSC):
            up = ub0 if sc % 2 == 0 else ub1
            sl = slice(sc * SC, (sc + 1) * SC)
            for c in range(KB):
                nc.tensor.matmul(
                    up[:],
                    at_sb[:, c, :],
                    x_sb[:, c, sl],
                    start=(c == 0),
                    stop=(KQ == 0 and c == KB - 1),
                )
            if KQ:
                nc.tensor.matmul(
                    up[:],
                    atq_sb[:],
                    xq_sb[:, :, sl],
                    start=False,
                    stop=True,
                    perf_mode=DRMODE,
                )
            nc.vector.tensor_copy(u_sb[:, sl], up[:])

        # ---- gate (after u: xsum reduces are off the critical path):
        # g[er] = sum_c gw[:,c,:]^T @ xsum[:,c] + gb  (gw pre-scaled 1/S)
        g_ps = ps_b.tile([128, 1], F32, tag="acc")
        for c in range(KC):
            nc.tensor.matmul(
                g_ps[:],
                gw_sb[:, c, :],
                xsum[:, c : c + 1],
                start=(c == 0),
                stop=(c == KC - 1),
            )
        nc.vector.tensor_add(g_sb[:], g_ps[:], gb_sb[:])

        # fold the gate into Bt: bts[er, o] = g[er] * Bt[er, o],
        # split across Vector and Scalar engines
        half = D_OUT // 2
        nc.scalar.activation(
            out=bts_sb[:, :half], in_=bt_sb[:, :half], func=Ident, scale=g_sb[:]
        )
        nc.vector.tensor_scalar_mul(bts_sb[:, half:], bt_sb[:, half:], g_sb[:])

        def lora_patch(oti, pool):
            laccs = []
            for _sc in range(NSC):
                lacc = pool.tile([128, SC], F32, tag="acc")
                laccs.append(lacc)
            osl = slice(oti * 128, (oti + 1) * 128)
            for sc in range(NSC):
                nc.tensor.matmul(
                    laccs[sc][:],
                    bts_sb[:, osl],
                    u_sb[:, sc * SC : (sc + 1) * SC],
                    start=True,
                    stop=True,
                )
            with nc.allow_low_precision(reason="lora add into fp16 out tile"):
                for sc in range(NSC):
                    sl = slice(sc * SC, (sc + 1) * SC)
                    nc.vector.tensor_add(
                        o_defer[oti][:, sl], o_defer[oti][:, sl], laccs[sc][:]
                    )
            ring = nc.sync if oti % 2 == 0 else nc.gpsimd
            ring.dma_start(out=outT[osl, :], in_=o_defer[oti][:])

        # ---- steady-state fused loop, lora patches for ot0/ot1 slotted
        # between ot2 and ot3 (bts/u are ready well before then)
        for ot in range(2, NOT):
            o_sb = opool.tile([128, S], FP16, tag="o")
            osl = slice(ot * 128, (ot + 1) * 128)
            accs = psum_group()
            if ot == NOT - 1:
                # drain the tail per s-chunk: lora -> copy-halves on both
                # engines -> store piece, so the last copies don't stack up
                # after the last matmul
                base_mms(accs, ot, close=False)
                for sc in range(NSC):
                    sl0 = slice(sc * SC, sc * SC + SC // 2)
                    sl1 = slice(sc * SC + SC // 2, (sc + 1) * SC)
                    nc.tensor.matmul(
                        accs[sc][:],
                        bts_sb[:, osl],
                        u_sb[:, sc * SC : (sc + 1) * SC],
                        start=False,
                        stop=True,
                    )
                    nc.vector.tensor_scalar_add(
                        o_sb[:, sl0], accs[sc][:, : SC // 2], bias_sb[:, ot : ot + 1]
                    )
                    nc.scalar.activation(
                        out=o_sb[:, sl1],
                        in_=accs[sc][:, SC // 2 :],
                        func=Ident,
                        bias=bias_sb[:, ot : ot + 1],
                        scale=1.0,
                    )
                    ring = nc.sync if sc % 2 == 0 else nc.gpsimd
                    ring.dma_start(
                        out=outT[osl, sc * SC : (sc + 1) * SC],
                        in_=o_sb[:, sc * SC : (sc + 1) * SC],
                    )
                continue
            base_mms(accs, ot, close=True)
            bias_copy(o_sb, accs, ot)
            if ot == 2:
                lora_patch(0, ps_b)
            elif ot == 3:
                lora_patch(1, ps_a)
            if ot == NOT - 2:
                for sc in range(NSC):
                    sl = slice(sc * SC, (sc + 1) * SC)
                    ring = nc.sync if (ot + sc) % 2 == 0 else nc.gpsimd
                    ring.dma_start(out=outT[osl, sl], in_=o_sb[:, sl])
            else:
                ring = nc.sync if ot % 2 == 0 else nc.gpsimd
                ring.dma_start(out=outT[osl, :], in_=o_sb[:])

    nc.compile()
    return nc


def _prep_in_maps(x, gate_w, gate_b, W, bias, lora_A, lora_B):
    f32 = np.float32
    x = np.asarray(x, f32)
    gate_w = np.asarray(gate_w, f32)
    gate_b = np.asarray(gate_b, f32)
    W = np.asarray(W, f32)
    bias = np.asarray(bias, f32)
    lora_A = np.asarray(lora_A, f32)
    lora_B = np.asarray(lora_B, f32)

    # WTb[ot, p, c, m] = W[ot*128 + m, c*128 + p]
    WTb = W.reshape(NOT, 128, KC, 128).transpose(0, 3, 2, 1)
    wte = np.ascontiguousarray(WTb[:NDEF, :, :KB, :].transpose(1, 2, 0, 3)).astype(
        NP_BF16
    )
    wp_b = np.ascontiguousarray(WTb[NDEF:, :, :KB, :].transpose(1, 2, 0, 3)).astype(
        NP_BF16
    )
    # AT[p, c, er] = A[er, c*128 + p]
    AT = lora_A.reshape(ER, D_IN).T.reshape(KC, 128, ER).transpose(1, 0, 2)
    at_b = np.ascontiguousarray(AT[:, :KB, :]).astype(NP_BF16)
    Bt = np.ascontiguousarray(lora_B.transpose(0, 2, 1).reshape(ER, D_OUT)).astype(
        NP_BF16
    )
    gwT = (
        (np.repeat(gate_w, R, axis=0).T / np.float32(S))
        .reshape(KC, 128, ER)
        .transpose(1, 0, 2)
    )
    gw = np.ascontiguousarray(gwT).astype(NP_BF16)
    gbr = np.ascontiguousarray(np.repeat(gate_b, R).reshape(ER, 1))
    bias_t = np.ascontiguousarray(bias.reshape(NOT, 128).T)

    shared = {
        "wte": wte,
        "wp_b": wp_b,
        "at_b": at_b,
        "bt": Bt,
        "gw": gw,
        "gb": gbr,
        "bias_t": bias_t,
    }
    if KQ:
        shared["wq_all"] = np.ascontiguousarray(
            WTb[:, :, KB:, :].transpose(1, 2, 0, 3)
        ).astype(NP_E4M3)
        shared["at_q"] = np.ascontiguousarray(AT[:, KB:, :]).astype(NP_E4M3)

    in_maps = []
    for b in range(NCORES):
        m = dict(shared)
        # p-major: xb[p, c, s] = x[s, b, c*128 + p], so each DMA line is a
        # full 4 KB row per partition
        xT = x[:, b, :].T.reshape(KC, 128, S)
        m["xb"] = np.ascontiguousarray(xT[:KB].transpose(1, 0, 2)).astype(NP_BF16)
        if KQ:
            m["xq"] = np.ascontiguousarray(xT[KB:].transpose(1, 0, 2)).astype(
                NP_E4M3
            )
        in_maps.append(m)
    return in_maps


def run(inputs, trace=False, trace_cores=None):
    """Build + run on 8 cores. Returns (out [S,B,D_OUT], BassKernelResults)."""
    in_maps = _prep_in_maps(**inputs)
    nc = _build_nc()
    kwargs = {}
    if trace:
        _register_axon_ntff_hook()
        kwargs = dict(trace=True, trace_cores=trace_cores or [0])
    res = bass_utils.run_bass_kernel_spmd(
        nc, in_maps, core_ids=list(range(NCORES)), **kwargs
    )
    out = np.empty((S, B, D_OUT), np.float32)
    for b in range(NCORES):
        out[:, b, :] = res.results[b]["outT"].T.astype(np.float32)
    return out, res


def _register_axon_ntff_hook():
    """antenv.axon_hooks is missing on this image; synthesize it so
    run_bass_kernel_spmd(trace=True) can reach the axon NTFF profiler."""
    import types

    try:
        from antenv.axon_hooks import get_axon_ntff_profile_hook  # noqa: F401

        return  # real module present
    except ImportError:
        pass
    try:
        from trn_agent_boot.trn_boot import _ntff_profile_via_ctypes
    except ImportError:
        return
    import antenv

    mod = types.ModuleType("antenv.axon_hooks")
    _state = {"hook": None}
    mod.set_axon_ntff_profile_hook = lambda h: _state.__setitem__("hook", h)
    mod.get_axon_ntff_profile_hook = lambda: _state["hook"]
    sys.modules["antenv.axon_hooks"] = mod
    antenv.axon_hooks = mod
    hook = _ntff_profile_via_ctypes("/opt/axon/libaxon_pjrt.so")
    if hook is not None:
        mod.set_axon_ntff_profile_hook(hook)


def kernel(**inputs) -> np.ndarray:
    out, _ = run(inputs, trace=False)
    return out


# revision 25
# speedup vs baseline: 1.0323x; 1.0066x over previous
"""LoRA-MoE fused kernel for 8x Trainium2 NeuronCores (Bass/Tile).

Math (per batch sample b, data-parallel across 8 cores):
    g_b    = gate_w @ mean_s(x_b) + gate_b                      # [E]
    out_b  = x_b @ W^T + ((x_b @ A^T) * g_rep) @ Bt + bias      # [S, D_OUT]
where A = lora_A reshaped [E*R, D_IN], Bt[(e,r), o] = lora_B[e, o, r],
g_rep[(e,r)] = g_b[e].  Merged per-sample weights are never materialized.

Precision strategy (gate: rel_err < 2e-2):
  - K-chunks 0..KB-1 of the base GEMM run in bf16 (measured matmul pitch
    217 ns vs 228 ns f32r, and half the HBM traffic of f32).
  - The last KQ=2 K-chunks run as ONE fp8-e4m3 DoubleRow matmul
    (K=256 in 221 ns -> 2x per-MAC vs bf16).  e4m3 on 2/8 of K costs
    rel_err 0.036*sqrt(2/8) ~= 1.8e-2; everything is deterministic
    (host-side rounding, f32 PSUM accumulation), so the margin holds.
  - LoRA path (u = A@x, then bts^T @ u) in bf16; its output contribution
    is ~1.6e-3 of the total, so its quantization error is negligible.
  - Output stored fp16 (halves the store traffic; ~1e-4 error).

Schedule: out^T tiles [o_tile=128 part, s_chunk=512 free]; contraction
on partitions.  ot0 runs base-only first (its PSUM groups stay open),
then gate + u while x finishes landing, then ot0's lora is appended and
ot1..31 run fused base+lora, alternating 4-bank PSUM groups.  PSUM->SBUF
copies add the bias and alternate Vector/Scalar engines.  DMA is spread
over the sync/scalar/gpsimd rings with fine-grained early x pieces.
"""

import sys

import numpy as np
import ml_dtypes

try:
    import concourse.bass  # noqa: F401
except ImportError:  # pragma: no cover - fallback for bare environments
    for _p in (
        "/root/.axon_site",
        "/root/.axon_site/_ro/trn_rl_repo",
        "/root/.axon_site/_ro/pypackages",
        "/opt/trn_rl_repo",
    ):
        if _p not in sys.path:
            sys.path.append(_p)

import concourse.bass as bass  # noqa: F401
import concourse.mybir as mybir
import concourse.tile as tile
from concourse import bacc, bass_utils

S, B, D_IN, D_OUT, E, R = 2048, 8, 1024, 4096, 8, 16
NCORES = 8
ER = E * R            # 128 (one partition dim worth of lora rows)
KC = D_IN // 128      # 8 contraction chunks
NOT = D_OUT // 128    # 32 output tiles
SC = 512              # s-chunk (one PSUM bank of f32)
NSC = S // SC         # 4

KQ = 2                # trailing K-chunks in fp8-e4m3 via DoubleRow
KB = KC - KQ          # leading K-chunks in bf16
NDEF = 2              # deferred o_tiles (base before the gate is ready)

F32 = mybir.dt.float32
BF16 = mybir.dt.bfloat16
FP16 = mybir.dt.float16
FP8 = mybir.dt.float8e4
DRMODE = mybir.MatmulPerfMode.DoubleRow

NP_BF16 = ml_dtypes.bfloat16
NP_E4M3 = ml_dtypes.float8_e4m3

Ident = mybir.ActivationFunctionType.Identity


def _build_nc(n_cores: int = NCORES):
    nc = bacc.Bacc(
        "TRN2", target_bir_lowering=False, debug=False, num_devices=n_cores
    )

    xb_d = nc.dram_tensor("xb", [128, KB, S], BF16, kind="ExternalInput").ap()
    wte_d = nc.dram_tensor(
        "wte", [128, KB, NDEF, 128], BF16, kind="ExternalInput"
    ).ap()
    wp_b = nc.dram_tensor(
        "wp_b", [128, KB, NOT - NDEF, 128], BF16, kind="ExternalInput"
    ).ap()
    at_b_d = nc.dram_tensor("at_b", [128, KB, ER], BF16, kind="ExternalInput").ap()
    bt_d = nc.dram_tensor("bt", [ER, D_OUT], BF16, kind="ExternalInput").ap()
    gw_d = nc.dram_tensor("gw", [128, KC, ER], BF16, kind="ExternalInput").ap()
    gb_d = nc.dram_tensor("gb", [ER, 1], F32, kind="ExternalInput").ap()
    bias_d = nc.dram_tensor("bias_t", [128, NOT], F32, kind="ExternalInput").ap()
    if KQ:
        xq_d = nc.dram_tensor("xq", [128, KQ, S], FP8, kind="ExternalInput").ap()
        wq_d = nc.dram_tensor(
            "wq_all", [128, KQ, NOT, 128], FP8, kind="ExternalInput"
        ).ap()
        at_q_d = nc.dram_tensor("at_q", [128, KQ, ER], FP8, kind="ExternalInput").ap()
    outT = nc.dram_tensor("outT", [D_OUT, S], FP16, kind="ExternalOutput").ap()

    with (
        tile.TileContext(nc) as tc,
        tc.tile_pool(name="singles", bufs=1) as singles,
        tc.tile_pool(name="opool", bufs=5) as opool,
        tc.tile_pool(name="ps_a", bufs=4, space="PSUM") as ps_a,
        tc.tile_pool(name="ps_b", bufs=4, space="PSUM") as ps_b,
    ):
        # ---- resident SBUF tensors
        x_sb = singles.tile([128, KB, S], BF16)
        wte_sb = singles.tile([128, KB, NDEF, 128], BF16)
        w_sb = singles.tile([128, KB, NOT - NDEF, 128], BF16)
        at_sb = singles.tile([128, KB, ER], BF16)
        bt_sb = singles.tile([128, D_OUT], BF16)
        gw_sb = singles.tile([128, KC, ER], BF16)
        gb_sb = singles.tile([128, 1], F32)
        bias_sb = singles.tile([128, NOT], F32)
        if KQ:
            xq_sb = singles.tile([128, KQ, S], FP8)
            wq_sb = singles.tile([128, KQ, NOT, 128], FP8)
            atq_sb = singles.tile([128, KQ, ER], FP8)
        xsum = singles.tile([128, KC], BF16)
        u_sb = singles.tile([128, S], BF16)
        g_sb = singles.tile([128, 1], F32)
        bts_sb = singles.tile([128, D_OUT], BF16)

        # ---- DMA kickoff.  Rings issue packets serially per queue and the
        # aggregate HBM read rate is the startup constraint, so the first
        # ~20us carries ONLY the working set of the deferred phase (wte + x
        # chunks + small gate tensors); all bulk weight loads queue strictly
        # behind them on their rings.  x chunk 0 is partition-split across
        # two rings for first-matmul latency.
        # Ring rates are uneven under contention and the gpsimd queue starts
        # ~4us late, so every x chunk is partition-split across two rings
        # round-robin: stream completion then tracks the aggregate HBM rate
        # instead of the slowest ring.  gpsimd gets only later-needed halves.
        # Whole-chunk x DMAs (halving every chunk across rings measured ~4.5us
        # slower).  Only x chunk 0 and wte are partition-split for first-
        # matmul latency.  xq leads the gpsimd queue (which starts ~4us late
        # but is otherwise idle): late xq cascades through xsum -> gate ->
        # bts into multi-us PE gaps.  wq/bt are split so they land by their
        # deadlines (~20us / ~26us) without racing the x stream.
        # CRITICAL: each engine recycles a small DMA-semaphore pool, so the
        # 5th+ dma_start on a queue BLOCKS that engine's instruction stream
        # until an earlier DMA completes.  The scalar queue therefore gets
        # only 5 early loads (its ACTIVATE copies must start by ~13us);
        # sync/gpsimd absorb all bulk, where trigger stalls are harmless.
        HP = 64
        nc.sync.dma_start(out=wte_sb[:HP], in_=wte_d[:HP])
        nc.sync.dma_start(out=x_sb[:, 0, 0:SC], in_=xb_d[:, 0, 0:SC])
        nc.sync.dma_start(out=x_sb[:, 0, 2 * SC : 3 * SC], in_=xb_d[:, 0, 2 * SC : 3 * SC])
        nc.sync.dma_start(out=x_sb[:, 1, :], in_=xb_d[:, 1, :])
        nc.sync.dma_start(out=x_sb[:, 4, :], in_=xb_d[:, 4, :])
        nc.sync.dma_start(out=gb_sb[:], in_=gb_d)
        nc.sync.dma_start(out=bias_sb[:], in_=bias_d)
        nc.scalar.dma_start(out=wte_sb[HP:], in_=wte_d[HP:])
        nc.scalar.dma_start(out=x_sb[:, 0, SC : 2 * SC], in_=xb_d[:, 0, SC : 2 * SC])
        nc.scalar.dma_start(out=x_sb[:, 0, 3 * SC :], in_=xb_d[:, 0, 3 * SC :])
        nc.scalar.dma_start(out=x_sb[:, 2, :], in_=xb_d[:, 2, :])
        nc.scalar.dma_start(out=x_sb[:, 5, :], in_=xb_d[:, 5, :])
        nc.scalar.dma_start(out=w_sb[:, 5], in_=wp_b[:, 5])
        if KQ:
            nc.gpsimd.dma_start(out=xq_sb[:], in_=xq_d[:])
        nc.gpsimd.dma_start(out=x_sb[:, 3, :], in_=xb_d[:, 3, :])
        if KQ:
            nc.gpsimd.dma_start(out=wq_sb[:], in_=wq_d[:])
        nc.gpsimd.dma_start(out=at_sb[:], in_=at_b_d)
        if KQ:
            nc.gpsimd.dma_start(out=atq_sb[:], in_=at_q_d)
        nc.gpsimd.dma_start(out=gw_sb[:], in_=gw_d)
        nc.gpsimd.dma_start(out=bt_sb[:], in_=bt_d)
        nc.gpsimd.dma_start(out=w_sb[:, 3], in_=wp_b[:, 3])
        for c in range(3):
            nc.sync.dma_start(out=w_sb[:, c], in_=wp_b[:, c])
        nc.sync.dma_start(out=w_sb[:, 4], in_=wp_b[:, 4])

        # ---- column sums for the gate, as x chunks land (all on Vector).
        # bf16 output is fine: DVE accumulates in f32 internally and the
        # gate path tolerates ~1e-2 relative error (its output share is
        # ~1.6e-3 of the total).
        with nc.allow_low_precision(reason="gate xsum, f32 internal accum"):
            for c in range(KQ):  # slow fp8 reduces first (xq lands early)
                nc.vector.reduce_sum(
                    out=xsum[:, KB + c : KB + c + 1],
                    in_=xq_sb[:, c, :],
                    axis=mybir.AxisListType.X,
                )
            for c in range(KB):
                nc.vector.reduce_sum(
                    out=xsum[:, c : c + 1],
                    in_=x_sb[:, c, :],
                    axis=mybir.AxisListType.X,
                )

        def wsl(ot, c):
            if ot < NDEF:
                return wte_sb[:, c, ot, :]
            return w_sb[:, c, ot - NDEF, :]

        def wqsl(ot):
            return wq_sb[:, :, ot, :]

        _ps_toggle = [0]

        def psum_group():
            pool = ps_a if _ps_toggle[0] % 2 == 0 else ps_b
            _ps_toggle[0] += 1
            accs = []
            for _sc in range(NSC):
                acc = pool.tile([128, SC], F32, tag="acc")
                accs.append(acc)
            return accs

        def base_mms(accs, ot, close):
            for c in range(KB):
                for sc in range(NSC):
                    nc.tensor.matmul(
                        accs[sc][:],
                        wsl(ot, c),
                        x_sb[:, c, sc * SC : (sc + 1) * SC],
                        start=(c == 0),
                        stop=False,
                    )
            if KQ:
                for sc in range(NSC):
                    nc.tensor.matmul(
                        accs[sc][:],
                        wqsl(ot),
                        xq_sb[:, :, sc * SC : (sc + 1) * SC],
                        start=False,
                        stop=False,
                        perf_mode=DRMODE,
                    )
            if close:
                lora_mms(accs, ot)

        def lora_mms(accs, ot):
            osl = slice(ot * 128, (ot + 1) * 128)
            for sc in range(NSC):
                nc.tensor.matmul(
                    accs[sc][:],
                    bts_sb[:, osl],
                    u_sb[:, sc * SC : (sc + 1) * SC],
                    start=False,
                    stop=True,
                )

        def bias_copy(o_sb, accs, ot):
            for sc in range(NSC):
                sl = slice(sc * SC, (sc + 1) * SC)
                if (ot + sc) % 2 == 0:
                    nc.vector.tensor_scalar_add(
                        o_sb[:, sl], accs[sc][:], bias_sb[:, ot : ot + 1]
                    )
                else:
                    nc.scalar.activation(
                        out=o_sb[:, sl],
                        in_=accs[sc][:],
                        func=Ident,
                        bias=bias_sb[:, ot : ot + 1],
                        scale=1.0,
                    )

        # ---- ot0 + ot1: base-only, interleaved chunk-major so the PE's
        # consumption rate (~1.74us/chunk for 8 MMs) matches the x DMA
        # arrival rate.  Groups close after the DR chunk; the lora part is
        # patched in later via separate PSUM groups + DVE adds.
        accs0 = psum_group()          # ps_a
        accs1 = psum_group()          # ps_b
        both = (accs0, accs1)
        for c in range(KB):
            for oti in range(2):
                for sc in range(NSC):
                    nc.tensor.matmul(
                        both[oti][sc][:],
                        wsl(oti, c),
                        x_sb[:, c, sc * SC : (sc + 1) * SC],
                        start=(c == 0),
                        stop=(KQ == 0 and c == KB - 1),
                    )
        if KQ:
            for oti in range(2):
                for sc in range(NSC):
                    nc.tensor.matmul(
                        both[oti][sc][:],
                        wqsl(oti),
                        xq_sb[:, :, sc * SC : (sc + 1) * SC],
                        start=False,
                        stop=True,
                        perf_mode=DRMODE,
                    )
        o_defer = []
        for oti in range(2):
            o_sb = opool.tile([128, S], FP16, tag="od")
            bias_copy(o_sb, both[oti], oti)
            o_defer.append(o_sb)

        # ---- u^T[er, s] = A @ x_b^T  (bf16 + fp8-DR chunks, 2 banks
        # ping-pong from the freed ps_a slots)
        ub0 = ps_a.tile([128, SC], F32, tag="acc")
        ub1 = ps_a.tile([128, SC], F32, tag="acc")
        for sc in range(NSC):
            up = ub0 if sc % 2 == 0 else ub1
            sl = slice(sc * SC, (sc + 1) * SC)
            for c in range(KB):
                nc.tensor.matmul(
                    up[:],
                    at_sb[:, c, :],
                    x_sb[:, c, sl],
                    start=(c == 0),
                    stop=(KQ == 0 and c == KB - 1),
                )
            if KQ:
                nc.tensor.matmul(
                    up[:],
                    atq_sb[:],
                    xq_sb[:, :, sl],
                    start=False,
                    stop=True,
                    perf_mode=DRMODE,
                )
            nc.vector.tensor_copy(u_sb[:, sl], up[:])

        # ---- gate (after u: xsum reduces are off the critical path):
        # g[er] = sum_c gw[:,c,:]^T @ xsum[:,c] + gb  (gw pre-scaled 1/S)
        g_ps = ps_b.tile([128, 1], F32, tag="acc")
        for c in range(KC):
            nc.tensor.matmul(
                g_ps[:],
                gw_sb[:, c, :],
                xsum[:, c : c + 1],
                start=(c == 0),
                stop=(c == KC - 1),
            )
        nc.vector.tensor_add(g_sb[:], g_ps[:], gb_sb[:])

        # fold the gate into Bt: bts[er, o] = g[er] * Bt[er, o],
        # split across Vector and Scalar engines
        half = D_OUT // 2
        nc.scalar.activation(
            out=bts_sb[:, :half], in_=bt_sb[:, :half], func=Ident, scale=g_sb[:]
        )
        nc.vector.tensor_scalar_mul(bts_sb[:, half:], bt_sb[:, half:], g_sb[:])

        def lora_patch(oti, pool):
            laccs = []
            for _sc in range(NSC):
                lacc = pool.tile([128, SC], F32, tag="acc")
                laccs.append(lacc)
            osl = slice(oti * 128, (oti + 1) * 128)
            for sc in range(NSC):
                nc.tensor.matmul(
                    laccs[sc][:],
                    bts_sb[:, osl],
                    u_sb[:, sc * SC : (sc + 1) * SC],
                    start=True,
                    stop=True,
                )
            with nc.allow_low_precision(reason="lora add into fp16 out tile"):
                for sc in range(NSC):
                    sl = slice(sc * SC, (sc + 1) * SC)
                    nc.vector.tensor_add(
                        o_defer[oti][:, sl], o_defer[oti][:, sl], laccs[sc][:]
                    )
            ring = nc.sync if oti % 2 == 0 else nc.gpsimd
            ring.dma_start(out=outT[osl, :], in_=o_defer[oti][:])

        # ---- steady-state fused loop, lora patches for ot0/ot1 slotted
        # between ot2 and ot3 (bts/u are ready well before then)
        for ot in range(2, NOT):
            o_sb = opool.tile([128, S], FP16, tag="o")
            osl = slice(ot * 128, (ot + 1) * 128)
            accs = psum_group()
            if ot == NOT - 1:
                # drain the tail per s-chunk: lora -> copy-halves on both
                # engines -> store piece, so the last copies don't stack up
                # after the last matmul
                base_mms(accs, ot, close=False)
                for sc in range(NSC):
                    sl0 = slice(sc * SC, sc * SC + SC // 2)
                    sl1 = slice(sc * SC + SC // 2, (sc + 1) * SC)
                    nc.tensor.matmul(
                        accs[sc][:],
                        bts_sb[:, osl],
                        u_sb[:, sc * SC : (sc + 1) * SC],
                        start=False,
                        stop=True,
                    )
                    nc.vector.tensor_scalar_add(
                        o_sb[:, sl0], accs[sc][:, : SC // 2], bias_sb[:, ot : ot + 1]
                    )
                    nc.scalar.activation(
                        out=o_sb[:, sl1],
                        in_=accs[sc][:, SC // 2 :],
                        func=Ident,
                        bias=bias_sb[:, ot : ot + 1],
                        scale=1.0,
                    )
                    ring = nc.sync if sc % 2 == 0 else nc.gpsimd
                    ring.dma_start(
                        out=outT[osl, sc * SC : (sc + 1) * SC],
                        in_=o_sb[:, sc * SC : (sc + 1) * SC],
                    )
                continue
            base_mms(accs, ot, close=True)
            bias_copy(o_sb, accs, ot)
            # patch pools chosen so the NEXT ot's psum_group comes from the
            # other pool and never waits on the patch's banks
            if ot == 2:
                lora_patch(0, ps_a)
            elif ot == 3:
                lora_patch(1, ps_b)
            if ot == NOT - 2:
                for sc in range(NSC):
                    sl = slice(sc * SC, (sc + 1) * SC)
                    ring = nc.sync if (ot + sc) % 2 == 0 else nc.gpsimd
                    ring.dma_start(out=outT[osl, sl], in_=o_sb[:, sl])
            else:
                ring = nc.sync if ot % 2 == 0 else nc.gpsimd
                ring.dma_start(out=outT[osl, :], in_=o_sb[:])

    nc.compile()
    return nc


def _prep_in_maps(x, gate_w, gate_b, W, bias, lora_A, lora_B):
    f32 = np.float32
    x = np.asarray(x, f32)
    gate_w = np.asarray(gate_w, f32)
    gate_b = np.asarray(gate_b, f32)
    W = np.asarray(W, f32)
    bias = np.asarray(bias, f32)
    lora_A = np.asarray(lora_A, f32)
    lora_B = np.asarray(lora_B, f32)

    # WTb[ot, p, c, m] = W[ot*128 + m, c*128 + p]
    WTb = W.reshape(NOT, 128, KC, 128).transpose(0, 3, 2, 1)
    wte = np.ascontiguousarray(WTb[:NDEF, :, :KB, :].transpose(1, 2, 0, 3)).astype(
        NP_BF16
    )
    wp_b = np.ascontiguousarray(WTb[NDEF:, :, :KB, :].transpose(1, 2, 0, 3)).astype(
        NP_BF16
    )
    # AT[p, c, er] = A[er, c*128 + p]
    AT = lora_A.reshape(ER, D_IN).T.reshape(KC, 128, ER).transpose(1, 0, 2)
    at_b = np.ascontiguousarray(AT[:, :KB, :]).astype(NP_BF16)
    Bt = np.ascontiguousarray(lora_B.transpose(0, 2, 1).reshape(ER, D_OUT)).astype(
        NP_BF16
    )
    gwT = (
        (np.repeat(gate_w, R, axis=0).T / np.float32(S))
        .reshape(KC, 128, ER)
        .transpose(1, 0, 2)
    )
    gw = np.ascontiguousarray(gwT).astype(NP_BF16)
    gbr = np.ascontiguousarray(np.repeat(gate_b, R).reshape(ER, 1))
    bias_t = np.ascontiguousarray(bias.reshape(NOT, 128).T)

    shared = {
        "wte": wte,
        "wp_b": wp_b,
        "at_b": at_b,
        "bt": Bt,
        "gw": gw,
        "gb": gbr,
        "bias_t": bias_t,
    }
    if KQ:
        shared["wq_all"] = np.ascontiguousarray(
            WTb[:, :, KB:, :].transpose(1, 2, 0, 3)
        ).astype(NP_E4M3)
        shared["at_q"] = np.ascontiguousarray(AT[:, KB:, :]).astype(NP_E4M3)

    in_maps = []
    for b in range(NCORES):
        m = dict(shared)
        # p-major: xb[p, c, s] = x[s, b, c*128 + p], so each DMA line is a
        # full 4 KB row per partition
        xT = x[:, b, :].T.reshape(KC, 128, S)
        m["xb"] = np.ascontiguousarray(xT[:KB].transpose(1, 0, 2)).astype(NP_BF16)
        if KQ:
            m["xq"] = np.ascontiguousarray(xT[KB:].transpose(1, 0, 2)).astype(
                NP_E4M3
            )
        in_maps.append(m)
    return in_maps


def run(inputs, trace=False, trace_cores=None):
    """Build + run on 8 cores. Returns (out [S,B,D_OUT], BassKernelResults)."""
    in_maps = _prep_in_maps(**inputs)
    nc = _build_nc()
    kwargs = {}
    if trace:
        _register_axon_ntff_hook()
        kwargs = dict(trace=True, trace_cores=trace_cores or [0])
    res = bass_utils.run_bass_kernel_spmd(
        nc, in_maps, core_ids=list(range(NCORES)), **kwargs
    )
    out = np.empty((S, B, D_OUT), np.float32)
    for b in range(NCORES):
        out[:, b, :] = res.results[b]["outT"].T.astype(np.float32)
    return out, res


def _register_axon_ntff_hook():
    """antenv.axon_hooks is missing on this image; synthesize it so
    run_bass_kernel_spmd(trace=True) can reach the axon NTFF profiler."""
    import types

    try:
        from antenv.axon_hooks import get_axon_ntff_profile_hook  # noqa: F401

        return  # real module present
    except ImportError:
        pass
    try:
        from trn_agent_boot.trn_boot import _ntff_profile_via_ctypes
    except ImportError:
        return
    import antenv

    mod = types.ModuleType("antenv.axon_hooks")
    _state = {"hook": None}
    mod.set_axon_ntff_profile_hook = lambda h: _state.__setitem__("hook", h)
    mod.get_axon_ntff_profile_hook = lambda: _state["hook"]
    sys.modules["antenv.axon_hooks"] = mod
    antenv.axon_hooks = mod
    hook = _ntff_profile_via_ctypes("/opt/axon/libaxon_pjrt.so")
    if hook is not None:
        mod.set_axon_ntff_profile_hook(hook)


def kernel(**inputs) -> np.ndarray:
    out, _ = run(inputs, trace=False)
    return out
